# revision 21
# baseline (speedup 1.0000x reference)
"""Trainium2 Bass kernel for nn_MultiHeadTransformerPosEmb.

Key observation: the module's output is `y[:, 0, :] @ wu.T` — only the CLS row
(position 0) of the transformer output feeds the unembedding. So per batch we
only need ONE attention query row per head; the [B,H,S,S] score tensor never
materializes (just [H, S] per batch).

Distribution over 8 NeuronCores:
 - batch-parallel attention: core b computes z_b = wo @ (attn row 0) + 2*y0 for
   batch b (embedding gather via one dma_gather, projections/scores on PE)
 - tiny AllGather of z (8 x 256 floats)
 - vocab-parallel unembed: core i computes out[:, 4096*i : 4096*(i+1)] against
   its 1/8 slice of wu (so wu's 32MB is read once total, not once per core)

Host-side prep is limited to index/layout transforms and constant tables.
"""
import math
from contextlib import ExitStack

import numpy as np
import ml_dtypes

import concourse.bass as bass
import concourse.tile as tile
import concourse.mybir as mybir
from concourse import bacc, bass_utils

F32 = mybir.dt.float32
BF16 = mybir.dt.bfloat16
I16 = mybir.dt.int16

NCORES = 8
B = 8
SEQ = 2048          # S + 1 (CLS prepended)
H = 256             # hidden
NH = 8              # heads
HD = 32             # head dim
V = 32001
VPAD = 32768
VSLICE = VPAD // NCORES   # 4096
NT = SEQ // 128           # 16 position tiles
SCALE = 1.0 / math.sqrt(HD)

_CACHE = {}


def _build():
    nc = bacc.Bacc("TRN2", target_bir_lowering=False, debug=False,
                   num_devices=NCORES)

    emb = nc.dram_tensor("emb", [V, H], BF16, kind="ExternalInput")
    idxs = nc.dram_tensor("idxs", [128, SEQ // 16], I16, kind="ExternalInput")
    pe = nc.dram_tensor("pe", [128, 2, SEQ], BF16, kind="ExternalInput")
    wq = nc.dram_tensor("wq", [128, 2, H], BF16, kind="ExternalInput")
    wk = nc.dram_tensor("wk", [128, 2, H], BF16, kind="ExternalInput")
    wv = nc.dram_tensor("wv", [128, 2, H], BF16, kind="ExternalInput")
    wo = nc.dram_tensor("wo", [128, 2, H], F32, kind="ExternalInput")
    maskc = nc.dram_tensor("maskc", [128, 2, NH], BF16, kind="ExternalInput")
    maskt = nc.dram_tensor("maskt", [NH, H], F32, kind="ExternalInput")
    ident = nc.dram_tensor("ident", [128, 128], F32, kind="ExternalInput")
    wu = nc.dram_tensor("wu", [128, 2, VSLICE], BF16, kind="ExternalInput")
    out = nc.dram_tensor("out", [B, VSLICE], F32, kind="ExternalOutput")

    with tile.TileContext(nc) as tc, ExitStack() as ctx:
        cp = ctx.enter_context(tc.tile_pool(name="const", bufs=1))
        wp = ctx.enter_context(tc.tile_pool(name="work", bufs=2))
        bigp = ctx.enter_context(tc.tile_pool(name="big", bufs=1))
        # PSUM budget is 8 banks; each pool below reserves bufs slots per tag,
        # so every tile in qp/op shares one tag.
        pp = ctx.enter_context(tc.tile_pool(name="vps", bufs=2, space="PSUM"))
        sp = ctx.enter_context(tc.tile_pool(name="sps", bufs=2, space="PSUM"))
        qp = ctx.enter_context(tc.tile_pool(name="qps", bufs=2, space="PSUM"))
        op = ctx.enter_context(tc.tile_pool(name="ops", bufs=2, space="PSUM"))
        dp = ctx.enter_context(tc.tile_pool(name="dram", bufs=1, space="DRAM"))

        # ---- constant loads -------------------------------------------------
        idx_sb = cp.tile([128, SEQ // 16], I16)
        nc.sync.dma_start(idx_sb[:], idxs[:])
        # ---- embedding gather (transposed, bf16) → yT -----------------------
        # yT_emb[p, c, i] = emb[tok[i]][c*128 + p]; two position-halves so the
        # pe-add and the v matmuls can start while the second half gathers.
        HSEQ = SEQ // 2
        yT_emb = [bigp.tile([128, 2, HSEQ], BF16, tag=f"yemb{h}",
                             name=f"yemb{h}") for h in range(2)]
        for h in range(2):
            nc.gpsimd.dma_gather(
                out_ap=yT_emb[h][:], in_ap=emb[:],
                idxs_ap=idx_sb[:, bass.ts(h, HSEQ // 16)],
                num_idxs=HSEQ, num_idxs_reg=HSEQ, elem_size=H, transpose=True,
                single_packet=False,
            )
        pe_sb = cp.tile([128, 2, SEQ], BF16)
        nc.sync.dma_start(pe_sb[:], pe[:])
        wv_sb = cp.tile([128, 2, H], BF16)
        nc.sync.dma_start(wv_sb[:], wv[:])
        wq_sb = cp.tile([128, 2, H], BF16)
        nc.sync.dma_start(wq_sb[:], wq[:])
        wk_sb = cp.tile([128, 2, H], BF16)
        nc.sync.dma_start(wk_sb[:], wk[:])
        wo_sb = cp.tile([128, 2, H], F32)
        nc.sync.dma_start(wo_sb[:], wo[:])
        maskc_sb = cp.tile([128, 2, NH], BF16)
        nc.sync.dma_start(maskc_sb[:], maskc[:])
        maskt_sb = cp.tile([NH, H], F32)
        nc.sync.dma_start(maskt_sb[:], maskt[:])
        id_sb = cp.tile([128, 128], F32)
        nc.sync.dma_start(id_sb[:], ident[:])
        wu_sb = cp.tile([128, 2, VSLICE], BF16)
        nc.sync.dma_start(wu_sb[:], wu[:])
        ones_sb = cp.tile([128, 1], F32)
        nc.vector.memset(ones_sb[:], 1.0)

        # yT = bf16(yT_emb) + pos_encoding^T (bf16); CLS column kept in fp32
        # separately for the exact residual 2*y0.
        yT = bigp.tile([128, 2, SEQ], BF16)
        for h in range(2):
            for c in range(2):
                nc.vector.tensor_tensor(
                    out=yT[:, c, bass.ts(h, HSEQ)], in0=yT_emb[h][:, c, :],
                    in1=pe_sb[:, c, bass.ts(h, HSEQ)],
                    op=mybir.AluOpType.add,
                )
        y0f = wp.tile([128, 2, 1], F32, tag="y0f")
        nc.vector.tensor_tensor(out=y0f[:], in0=yT_emb[0][:, :, 0:1],
                                in1=pe_sb[:, :, 0:1], op=mybir.AluOpType.add)

        # ---- v = y @ Wv.T, [pos, hd] layout, + ones column for softmax sum --
        v_all = bigp.tile([128, NT, H + 1], BF16)
        nc.vector.memset(v_all[:, :, H:H + 1], 1.0)
        for t in range(NT):
            vps = pp.tile([128, H], F32)
            for c in range(2):
                nc.tensor.matmul(vps[:], lhsT=yT[:, c, bass.ts(t, 128)],
                                 rhs=wv_sb[:, c, :], start=(c == 0), stop=(c == 1))
            if t % 2 == 0:
                nc.vector.tensor_copy(out=v_all[:, t, 0:H], in_=vps[:])
            else:
                nc.scalar.copy(v_all[:, t, 0:H], vps[:])

        # ---- q0 (scaled), block-diag columns bd -----------------------------
        bd_sb = wp.tile([128, 2, NH], BF16, tag="bd")
        for m in range(2):
            qps = qp.tile([128, NH], F32, tag="small")
            for c in range(2):
                nc.tensor.matmul(qps[:, 0:1], lhsT=wq_sb[:, c, bass.ts(m, 128)],
                                 rhs=yT[:, c, 0:1], start=(c == 0), stop=(c == 1))
            q0c = wp.tile([128, 1], BF16, tag="q0c")
            nc.scalar.mul(q0c[:], qps[:, 0:1], SCALE)
            nc.vector.tensor_tensor(out=bd_sb[:, m, :],
                                    in0=q0c[:].to_broadcast([128, NH]),
                                    in1=maskc_sb[:, m, :],
                                    op=mybir.AluOpType.mult)

        # ---- qk = Wk_flat.T @ bd  (fuses k-projection into score matmul) ----
        qk_sb = wp.tile([128, 2, NH], BF16, tag="qk")
        for m in range(2):
            qkps = qp.tile([128, NH], F32, tag="small")
            for c in range(2):
                nc.tensor.matmul(qkps[:], lhsT=wk_sb[:, c, bass.ts(m, 128)],
                                 rhs=bd_sb[:, c, :], start=(c == 0), stop=(c == 1))
            nc.vector.tensor_copy(out=qk_sb[:, m, :], in_=qkps[:])

        # ---- scores (transposed) + exp: aT[pos, head] = exp(yT.T @ qk) ------
        # 4 position-tiles share one PSUM tile so a single Exp covers them,
        # cutting the PE->ACT ping-pong count 4x.
        aT_all = bigp.tile([128, NT, NH], BF16)
        for g in range(NT // 4):
            sps = sp.tile([128, 4, NH], F32)
            for j in range(4):
                t = 4 * g + j
                for c in range(2):
                    nc.tensor.matmul(sps[:, j, :],
                                     lhsT=yT[:, c, bass.ts(t, 128)],
                                     rhs=qk_sb[:, c, :],
                                     start=(c == 0), stop=(c == 1))
            nc.scalar.activation(out=aT_all[:, bass.ts(g, 4), :], in_=sps[:],
                                 func=mybir.ActivationFunctionType.Exp)

        # ---- o0T[head, hd (+denom)] = sum_pos aT * v ------------------------
        o0ps = op.tile([NH, H + 1], F32, tag="acc")
        for t in range(NT):
            nc.tensor.matmul(o0ps[:], lhsT=aT_all[:, t, :], rhs=v_all[:, t, :],
                             start=(t == 0), stop=(t == NT - 1))

        # select diagonal blocks (mask), fold to column, then apply the
        # softmax denominator per-head on the folded column.
        recip = wp.tile([NH, 1], F32, tag="recip")
        nc.vector.reciprocal(recip[:], o0ps[:, H:H + 1])
        rexp_sb = wp.tile([128, 2, 1], F32, tag="rexp")
        for c in range(2):
            rexps = qp.tile([128, NH], F32, tag="small")
            nc.tensor.matmul(rexps[:, 0:1], lhsT=maskt_sb[:, bass.ts(c, 128)],
                             rhs=recip[:], start=True, stop=True)
            nc.vector.tensor_copy(out=rexp_sb[:, c, :], in_=rexps[:, 0:1])
        o0m = wp.tile([NH, H], F32, tag="o0m")
        nc.vector.tensor_tensor(out=o0m[:], in0=o0ps[:, 0:H], in1=maskt_sb[:],
                                op=mybir.AluOpType.mult)
        oc_sb = wp.tile([128, 2, 1], F32, tag="oc")
        for c in range(2):
            ocps = qp.tile([128, NH], F32, tag="small")
            nc.tensor.matmul(ocps[:, 0:1], lhsT=o0m[:, bass.ts(c, 128)],
                             rhs=ones_sb[0:NH, :], start=True, stop=True)
            nc.vector.tensor_tensor(out=oc_sb[:, c, :], in0=ocps[:, 0:1],
                                    in1=rexp_sb[:, c, :],
                                    op=mybir.AluOpType.mult)

        # ---- z = wo @ o0 + 2*y0, transposed to a [1, 256] row ---------------
        zrow = wp.tile([1, H], F32, tag="zrow")
        zrall = qp.tile([1, H], F32, tag="small")
        for m in range(2):
            zps = qp.tile([128, NH], F32, tag="small")
            for c in range(2):
                nc.tensor.matmul(zps[:, 0:1], lhsT=wo_sb[:, c, bass.ts(m, 128)],
                                 rhs=oc_sb[:, c, :], start=(c == 0), stop=(c == 1))
            y2 = wp.tile([128, 1], F32, tag="y2")
            nc.vector.tensor_scalar_mul(y2[:], y0f[:, m, :], 2.0)
            zc = wp.tile([128, 1], F32, tag="zc")
            nc.vector.tensor_tensor(out=zc[:], in0=zps[:, 0:1], in1=y2[:],
                                    op=mybir.AluOpType.add)
            nc.tensor.matmul(zrall[0:1, bass.ts(m, 128)], lhsT=zc[:],
                             rhs=id_sb[:], start=True, stop=True)
            if m == 1:
                nc.vector.tensor_copy(out=zrow[:], in_=zrall[:])
                # keep PE warm across the AllGather (p-state ramp)
                for _ in range(10):
                    wps = qp.tile([1, 128], F32, tag="small")
                    nc.tensor.matmul(wps[:], lhsT=zc[:], rhs=id_sb[:],
                                     start=True, stop=True)

        # ---- AllGather z across the 8 cores ---------------------------------
        zb = dp.tile([1, H], F32)
        zg = dp.tile([B, H], F32)
        nc.sync.dma_start(zb[:], zrow[:])
        nc.gpsimd.collective_compute(
            "AllGather", mybir.AluOpType.bypass,
            replica_groups=[list(range(NCORES))],
            ins=[zb[:].opt()], outs=[zg[:].opt()],
        )
        zg_sb = wp.tile([B, H], F32, tag="zg")
        nc.sync.dma_start(zg_sb[:], zg[:])
        zt_sb = wp.tile([128, 2, B], BF16, tag="zt")
        for c in range(2):
            ztps = qp.tile([128, NH], F32, tag="small")
            nc.tensor.matmul(ztps[:], lhsT=zg_sb[:, bass.ts(c, 128)],
                             rhs=id_sb[0:B, 0:B], start=True, stop=True)
            nc.vector.tensor_copy(out=zt_sb[:, c, :], in_=ztps[:])

        # ---- unembed: out[b, j] = Z @ wu_slice.T ----------------------------
        osb = wp.tile([B, VSLICE], F32, tag="osb")
        for nch in range(VSLICE // 512):
            ups = (op if nch % 2 == 0 else pp).tile(
                [B, 512], F32, tag="acc" if nch % 2 == 0 else "vps")
            for c in range(2):
                nc.tensor.matmul(ups[:], lhsT=zt_sb[:, c, :],
                                 rhs=wu_sb[:, c, bass.ts(nch, 512)],
                                 start=(c == 0), stop=(c == 1))
            if nch % 2 == 0:
                nc.vector.tensor_copy(out=osb[:, bass.ts(nch, 512)], in_=ups[:])
            else:
                nc.scalar.copy(osb[:, bass.ts(nch, 512)], ups[:])
        nc.sync.dma_start(out[:], osb[:])

    nc.finalize()
    return nc


def _pos_encoding_np():
    pos = np.arange(SEQ, dtype=np.float32)[:, None]
    div = np.exp(np.arange(0, H, 2, dtype=np.float32)
                 * np.float32(-(math.log(10000.0) / H)))
    ang = pos * div[None, :]
    pe = np.zeros((SEQ, H), dtype=np.float32)
    pe[:, 0::2] = np.sin(ang)
    pe[:, 1::2] = np.cos(ang)
    return pe


def _part_chunk(a2d):
    """[256, N] -> [128, 2, N] with [p, c, :] = a2d[c*128 + p]."""
    n = a2d.shape[1]
    return np.ascontiguousarray(a2d.reshape(2, 128, n).transpose(1, 0, 2))


def prepare_in_maps(x, emb_w, wq, wk, wv, wo, wu):
    x = np.asarray(x)
    emb_w = np.asarray(emb_w, dtype=np.float32)
    wq = np.asarray(wq, dtype=np.float32)
    wk = np.asarray(wk, dtype=np.float32)
    wv = np.asarray(wv, dtype=np.float32)
    wo = np.asarray(wo, dtype=np.float32)
    wu = np.asarray(wu, dtype=np.float32)

    tok = np.concatenate(
        [np.full((B, 1), 2, dtype=np.int64), x], axis=1).astype(np.int16)

    emb_host = emb_w.astype(ml_dtypes.bfloat16)
    peT = _part_chunk(_pos_encoding_np().T).astype(ml_dtypes.bfloat16)                      # [128,2,SEQ]
    wq_host = _part_chunk(wq.reshape(H, H).T).astype(ml_dtypes.bfloat16)                    # [p,c,hd]
    wk_host = _part_chunk(wk.reshape(H, H)).astype(ml_dtypes.bfloat16)                      # [p,c,h]
    wv_host = _part_chunk(wv.reshape(H, H).T).astype(ml_dtypes.bfloat16)                    # [p,c,hd]
    wo_host = _part_chunk(wo.T)                                  # [p,c,j]
    hd_idx = np.arange(H) // HD
    maskc_host = np.ascontiguousarray(
        (hd_idx.reshape(2, 128)[:, :, None] == np.arange(NH)[None, None, :])
        .astype(ml_dtypes.bfloat16).transpose(1, 0, 2))          # [128,2,8]
    maskt_host = (hd_idx[None, :] == np.arange(NH)[:, None]).astype(np.float32)
    ident_host = np.eye(128, dtype=np.float32)
    wu_pad = np.zeros((VPAD, H), dtype=np.float32)
    wu_pad[:V] = wu

    in_maps = []
    for core in range(NCORES):
        tb = tok[core]
        idx_t = np.tile(np.ascontiguousarray(tb.reshape(SEQ // 16, 16).T),
                        (8, 1))                                  # [128, 128]
        wu_host = _part_chunk(
            np.ascontiguousarray(
                wu_pad[VSLICE * core: VSLICE * (core + 1)].T)).astype(
                    ml_dtypes.bfloat16)
        in_maps.append({
            "emb": emb_host, "idxs": idx_t, "pe": peT,
            "wq": wq_host, "wk": wk_host, "wv": wv_host, "wo": wo_host,
            "maskc": maskc_host, "maskt": maskt_host, "ident": ident_host,
            "wu": wu_host,
        })
    return in_maps


def get_nc():
    if "nc" not in _CACHE:
        _CACHE["nc"] = _build()
    return _CACHE["nc"]


def assemble(results):
    full = np.concatenate([results[i]["out"] for i in range(NCORES)],
                          axis=1)[:, :V]
    return np.ascontiguousarray(full.astype(np.float32))


def kernel(x, emb_w, wq, wk, wv, wo, wu):
    nc = get_nc()
    in_maps = prepare_in_maps(x, emb_w, wq, wk, wv, wo, wu)
    res = bass_utils.run_bass_kernel_spmd(
        nc, in_maps, core_ids=list(range(NCORES)))
    return assemble(res.results)


# revision 28
# speedup vs baseline: 1.0392x; 1.0392x over previous
"""Trainium2 Bass kernel for nn_MultiHeadTransformerPosEmb.

Key observation: the module's output is `y[:, 0, :] @ wu.T` — only the CLS row
(position 0) of the transformer output feeds the unembedding. So per batch we
only need ONE attention query row per head; the [B,H,S,S] score tensor never
materializes (just [H, S] per batch).

Distribution over 8 NeuronCores:
 - batch-parallel attention: core b computes z_b = wo @ (attn row 0) + 2*y0 for
   batch b (embedding gather via one dma_gather, projections/scores on PE)
 - tiny AllGather of z (8 x 256 floats)
 - vocab-parallel unembed: core i computes out[:, 4096*i : 4096*(i+1)] against
   its 1/8 slice of wu (so wu's 32MB is read once total, not once per core)

Host-side prep is limited to index/layout transforms and constant tables.
"""
import math
from contextlib import ExitStack

import numpy as np
import ml_dtypes

import concourse.bass as bass
import concourse.tile as tile
import concourse.mybir as mybir
from concourse import bacc, bass_utils

F32 = mybir.dt.float32
BF16 = mybir.dt.bfloat16
I16 = mybir.dt.int16

NCORES = 8
B = 8
SEQ = 2048          # S + 1 (CLS prepended)
H = 256             # hidden
NH = 8              # heads
HD = 32             # head dim
V = 32001
VPAD = 32768
VSLICE = VPAD // NCORES   # 4096
NT = SEQ // 128           # 16 position tiles
SCALE = 1.0 / math.sqrt(HD)
NWARM = 90

_CACHE = {}


def _build():
    nc = bacc.Bacc("TRN2", target_bir_lowering=False, debug=False,
                   num_devices=NCORES)

    emb = nc.dram_tensor("emb", [V, H], BF16, kind="ExternalInput")
    idxs = nc.dram_tensor("idxs", [128, SEQ // 16], I16, kind="ExternalInput")
    pe = nc.dram_tensor("pe", [128, 2, SEQ], BF16, kind="ExternalInput")
    wq = nc.dram_tensor("wq", [128, 2, H], BF16, kind="ExternalInput")
    wk = nc.dram_tensor("wk", [128, 2, H], BF16, kind="ExternalInput")
    wv = nc.dram_tensor("wv", [128, 2, H], BF16, kind="ExternalInput")
    wo = nc.dram_tensor("wo", [128, 2, H], F32, kind="ExternalInput")
    maskc = nc.dram_tensor("maskc", [128, 2, NH], BF16, kind="ExternalInput")
    maskt = nc.dram_tensor("maskt", [NH, H], F32, kind="ExternalInput")
    ident = nc.dram_tensor("ident", [128, 128], F32, kind="ExternalInput")
    wu = nc.dram_tensor("wu", [128, 2, VSLICE], BF16, kind="ExternalInput")
    out = nc.dram_tensor("out", [B, VSLICE], F32, kind="ExternalOutput")

    with tile.TileContext(nc) as tc, ExitStack() as ctx:
        cp = ctx.enter_context(tc.tile_pool(name="const", bufs=1))
        wp = ctx.enter_context(tc.tile_pool(name="work", bufs=2))
        bigp = ctx.enter_context(tc.tile_pool(name="big", bufs=1))
        # PSUM budget is 8 banks; each pool below reserves bufs slots per tag,
        # so every tile in qp/op shares one tag.
        pp = ctx.enter_context(tc.tile_pool(name="vps", bufs=2, space="PSUM"))
        sp = ctx.enter_context(tc.tile_pool(name="sps", bufs=2, space="PSUM"))
        qp = ctx.enter_context(tc.tile_pool(name="qps", bufs=2, space="PSUM"))
        op = ctx.enter_context(tc.tile_pool(name="ops", bufs=2, space="PSUM"))
        dp = ctx.enter_context(tc.tile_pool(name="dram", bufs=1, space="DRAM"))

        # ---- constant loads -------------------------------------------------
        idx_sb = cp.tile([128, SEQ // 16], I16)
        nc.sync.dma_start(idx_sb[:], idxs[:])
        pe_sb = cp.tile([128, 2, SEQ], BF16)
        nc.sync.dma_start(pe_sb[:, :, 0:SEQ // 2], pe[:, :, 0:SEQ // 2])
        nc.sync.dma_start(pe_sb[:, :, SEQ // 2:], pe[:, :, SEQ // 2:])
        wv_sb = cp.tile([128, 2, H], BF16)
        nc.sync.dma_start(wv_sb[:], wv[:])
        maskc_sb = cp.tile([128, 2, NH], BF16)
        nc.sync.dma_start(maskc_sb[:], maskc[:])
        # ---- embedding gather (transposed, bf16) → yT -----------------------
        # yT_emb[p, c, i] = emb[tok[i]][c*128 + p]; two position-halves so the
        # pe-add and the v matmuls can start while the second half gathers.
        HSEQ = SEQ // 2
        yT_emb = [bigp.tile([128, 2, HSEQ], BF16, tag=f"yemb{h}",
                             name=f"yemb{h}") for h in range(2)]
        for h in range(2):
            nc.gpsimd.dma_gather(
                out_ap=yT_emb[h][:], in_ap=emb[:],
                idxs_ap=idx_sb[:, bass.ts(h, HSEQ // 16)],
                num_idxs=HSEQ, num_idxs_reg=HSEQ, elem_size=H, transpose=True,
                single_packet=False,
            )
        wq_sb = cp.tile([128, 2, H], BF16)
        nc.sync.dma_start(wq_sb[:], wq[:])
        wk_sb = cp.tile([128, 2, H], BF16)
        nc.sync.dma_start(wk_sb[:], wk[:])
        wo_sb = cp.tile([128, 2, H], F32)
        nc.sync.dma_start(wo_sb[:], wo[:])
        maskt_sb = cp.tile([NH, H], F32)
        nc.sync.dma_start(maskt_sb[:], maskt[:])
        id_sb = cp.tile([128, 128], F32)
        nc.sync.dma_start(id_sb[:], ident[:])
        wu_sb = cp.tile([128, 2, VSLICE], BF16)
        nc.sync.dma_start(wu_sb[:], wu[:])
        ones_sb = cp.tile([128, 1], F32)
        nc.vector.memset(ones_sb[:], 1.0)

        # yT = bf16(yT_emb) + pos_encoding^T (bf16); CLS column kept in fp32
        # separately for the exact residual 2*y0.
        yT = bigp.tile([128, 2, SEQ], BF16)
        y0f = wp.tile([128, 2, 1], F32, tag="y0f")
        nc.vector.tensor_tensor(out=y0f[:], in0=yT_emb[0][:, :, 0:1],
                                in1=pe_sb[:, :, 0:1], op=mybir.AluOpType.add)
        for h in range(2):
            for c in range(2):
                nc.vector.tensor_tensor(
                    out=yT[:, c, bass.ts(h, HSEQ)], in0=yT_emb[h][:, c, :],
                    in1=pe_sb[:, c, bass.ts(h, HSEQ)],
                    op=mybir.AluOpType.add,
                )

        # ---- q0 (scaled), block-diag columns bd -----------------------------
        bd_sb = wp.tile([128, 2, NH], BF16, tag="bd")
        for m in range(2):
            qps = qp.tile([128, NH], F32, tag="small")
            for c in range(2):
                nc.tensor.matmul(qps[:, 0:1], lhsT=wq_sb[:, c, bass.ts(m, 128)],
                                 rhs=yT[:, c, 0:1], start=(c == 0), stop=(c == 1))
            q0c = wp.tile([128, 1], BF16, tag="q0c")
            nc.scalar.mul(q0c[:], qps[:, 0:1], SCALE)
            nc.vector.tensor_tensor(out=bd_sb[:, m, :],
                                    in0=q0c[:].to_broadcast([128, NH]),
                                    in1=maskc_sb[:, m, :],
                                    op=mybir.AluOpType.mult)

        # ---- qk = Wk_flat.T @ bd  (fuses k-projection into score matmul) ----
        qk_sb = wp.tile([128, 2, NH], BF16, tag="qk")
        for m in range(2):
            qkps = qp.tile([128, NH], F32, tag="small")
            for c in range(2):
                nc.tensor.matmul(qkps[:], lhsT=wk_sb[:, c, bass.ts(m, 128)],
                                 rhs=bd_sb[:, c, :], start=(c == 0), stop=(c == 1))
            nc.vector.tensor_copy(out=qk_sb[:, m, :], in_=qkps[:])

        # ---- v = y @ Wv.T, [pos, hd] layout, + ones column for softmax sum --
        v_all = bigp.tile([128, NT, H + 1], BF16)
        nc.vector.memset(v_all[:, :, H:H + 1], 1.0)
        for pair in range(NT // 2):
            vps = pp.tile([128, 2, H], F32)
            for j in range(2):
                t = 2 * pair + j
                for c in range(2):
                    nc.tensor.matmul(vps[:, j, :],
                                     lhsT=yT[:, c, bass.ts(t, 128)],
                                     rhs=wv_sb[:, c, :],
                                     start=(c == 0), stop=(c == 1))
            dst = v_all[:, bass.ts(pair, 2), 0:H]
            if pair < 2 or pair >= 6:
                nc.scalar.copy(dst, vps[:])
            else:
                nc.vector.tensor_copy(out=dst, in_=vps[:])

        # ---- scores (transposed) + exp: aT[pos, head] = exp(yT.T @ qk) ------
        # 4 position-tiles share one PSUM tile so a single Exp covers them,
        # cutting the PE->ACT ping-pong count 4x.
        aT_all = bigp.tile([128, NT, NH], BF16)
        for g in range(NT // 4):
            sps = sp.tile([128, 4, NH], F32)
            for j in range(4):
                t = 4 * g + j
                for c in range(2):
                    nc.tensor.matmul(sps[:, j, :],
                                     lhsT=yT[:, c, bass.ts(t, 128)],
                                     rhs=qk_sb[:, c, :],
                                     start=(c == 0), stop=(c == 1))
            nc.scalar.activation(out=aT_all[:, bass.ts(g, 4), :], in_=sps[:],
                                 func=mybir.ActivationFunctionType.Exp)

        # ---- o0T[head, hd (+denom)] = sum_pos aT * v ------------------------
        o0ps = op.tile([NH, H + 1], F32, tag="acc")
        for t in range(NT):
            nc.tensor.matmul(o0ps[:], lhsT=aT_all[:, t, :], rhs=v_all[:, t, :],
                             start=(t == 0), stop=(t == NT - 1))

        # select diagonal blocks (mask), fold to column, then apply the
        # softmax denominator per-head on the folded column.
        recip = wp.tile([NH, 1], F32, tag="recip")
        nc.vector.reciprocal(recip[:], o0ps[:, H:H + 1])
        rexp_sb = wp.tile([128, 2, 1], F32, tag="rexp")
        for c in range(2):
            rexps = qp.tile([128, NH], F32, tag="small")
            nc.tensor.matmul(rexps[:, 0:1], lhsT=maskt_sb[:, bass.ts(c, 128)],
                             rhs=recip[:], start=True, stop=True)
            nc.vector.tensor_copy(out=rexp_sb[:, c, :], in_=rexps[:, 0:1])
        o0m = wp.tile([NH, H], F32, tag="o0m")
        nc.vector.tensor_tensor(out=o0m[:], in0=o0ps[:, 0:H], in1=maskt_sb[:],
                                op=mybir.AluOpType.mult)
        oc_sb = wp.tile([128, 2, 1], F32, tag="oc")
        for c in range(2):
            ocps = qp.tile([128, NH], F32, tag="small")
            nc.tensor.matmul(ocps[:, 0:1], lhsT=o0m[:, bass.ts(c, 128)],
                             rhs=ones_sb[0:NH, :], start=True, stop=True)
            nc.vector.tensor_tensor(out=oc_sb[:, c, :], in0=ocps[:, 0:1],
                                    in1=rexp_sb[:, c, :],
                                    op=mybir.AluOpType.mult)

        # ---- z = wo @ o0 + 2*y0, transposed to a [1, 256] row ---------------
        zrow = wp.tile([1, H], F32, tag="zrow")
        zrall = qp.tile([1, H], F32, tag="small")
        for m in range(2):
            zps = qp.tile([128, NH], F32, tag="small")
            for c in range(2):
                nc.tensor.matmul(zps[:, 0:1], lhsT=wo_sb[:, c, bass.ts(m, 128)],
                                 rhs=oc_sb[:, c, :], start=(c == 0), stop=(c == 1))
            y2 = wp.tile([128, 1], F32, tag="y2")
            nc.vector.tensor_scalar_mul(y2[:], y0f[:, m, :], 2.0)
            zc = wp.tile([128, 1], F32, tag="zc")
            nc.vector.tensor_tensor(out=zc[:], in0=zps[:, 0:1], in1=y2[:],
                                    op=mybir.AluOpType.add)
            nc.tensor.matmul(zrall[0:1, bass.ts(m, 128)], lhsT=zc[:],
                             rhs=id_sb[:], start=True, stop=True)
            if m == 1:
                nc.vector.tensor_copy(out=zrow[:], in_=zrall[:])
                # keep PE warm across the AllGather (p-state ramp): a chain of
                # throwaway transposes bridges the otherwise-idle window so the
                # unembed matmuls run at full clock.
                for _ in range(NWARM):
                    wps = qp.tile([1, 128], F32, tag="small")
                    nc.tensor.matmul(wps[:], lhsT=zc[:], rhs=id_sb[:],
                                     start=True, stop=True)

        # ---- AllGather z across the 8 cores ---------------------------------
        zb = dp.tile([1, H], F32)
        zg = dp.tile([B, H], F32)
        nc.sync.dma_start(zb[:], zrow[:])
        nc.gpsimd.collective_compute(
            "AllGather", mybir.AluOpType.bypass,
            replica_groups=[list(range(NCORES))],
            ins=[zb[:].opt()], outs=[zg[:].opt()],
        )
        zg_sb = wp.tile([B, H], F32, tag="zg")
        nc.sync.dma_start(zg_sb[:], zg[:])
        zt_sb = wp.tile([128, 2, B], BF16, tag="zt")
        for c in range(2):
            ztps = qp.tile([128, NH], F32, tag="small")
            nc.tensor.matmul(ztps[:], lhsT=zg_sb[:, bass.ts(c, 128)],
                             rhs=id_sb[0:B, 0:B], start=True, stop=True)
            if c == 0:
                nc.vector.tensor_copy(out=zt_sb[:, c, :], in_=ztps[:])
            else:
                nc.scalar.copy(zt_sb[:, c, :], ztps[:])

        # ---- unembed: out[b, j] = Z @ wu_slice.T ----------------------------
        osb = wp.tile([B, VSLICE], F32, tag="osb")
        for nch in range(VSLICE // 512):
            ups = (op if nch % 2 == 0 else pp).tile(
                [B, 512], F32, tag="acc" if nch % 2 == 0 else "vps")
            for c in range(2):
                nc.tensor.matmul(ups[:], lhsT=zt_sb[:, c, :],
                                 rhs=wu_sb[:, c, bass.ts(nch, 512)],
                                 start=(c == 0), stop=(c == 1))
            if nch % 2 == 0:
                nc.vector.tensor_copy(out=osb[:, bass.ts(nch, 512)], in_=ups[:])
            else:
                nc.scalar.copy(osb[:, bass.ts(nch, 512)], ups[:])
        nc.sync.dma_start(out[:, 0:VSLICE // 2], osb[:, 0:VSLICE // 2])
        nc.sync.dma_start(out[:, VSLICE // 2:], osb[:, VSLICE // 2:])

    nc.finalize()
    return nc


def _pos_encoding_np():
    pos = np.arange(SEQ, dtype=np.float32)[:, None]
    div = np.exp(np.arange(0, H, 2, dtype=np.float32)
                 * np.float32(-(math.log(10000.0) / H)))
    ang = pos * div[None, :]
    pe = np.zeros((SEQ, H), dtype=np.float32)
    pe[:, 0::2] = np.sin(ang)
    pe[:, 1::2] = np.cos(ang)
    return pe


def _part_chunk(a2d):
    """[256, N] -> [128, 2, N] with [p, c, :] = a2d[c*128 + p]."""
    n = a2d.shape[1]
    return np.ascontiguousarray(a2d.reshape(2, 128, n).transpose(1, 0, 2))


def prepare_in_maps(x, emb_w, wq, wk, wv, wo, wu):
    x = np.asarray(x)
    emb_w = np.asarray(emb_w, dtype=np.float32)
    wq = np.asarray(wq, dtype=np.float32)
    wk = np.asarray(wk, dtype=np.float32)
    wv = np.asarray(wv, dtype=np.float32)
    wo = np.asarray(wo, dtype=np.float32)
    wu = np.asarray(wu, dtype=np.float32)

    tok = np.concatenate(
        [np.full((B, 1), 2, dtype=np.int64), x], axis=1).astype(np.int16)

    emb_host = emb_w.astype(ml_dtypes.bfloat16)
    peT = _part_chunk(_pos_encoding_np().T).astype(ml_dtypes.bfloat16)                      # [128,2,SEQ]
    wq_host = _part_chunk(wq.reshape(H, H).T).astype(ml_dtypes.bfloat16)                    # [p,c,hd]
    wk_host = _part_chunk(wk.reshape(H, H)).astype(ml_dtypes.bfloat16)                      # [p,c,h]
    wv_host = _part_chunk(wv.reshape(H, H).T).astype(ml_dtypes.bfloat16)                    # [p,c,hd]
    wo_host = _part_chunk(wo.T)                                  # [p,c,j]
    hd_idx = np.arange(H) // HD
    maskc_host = np.ascontiguousarray(
        (hd_idx.reshape(2, 128)[:, :, None] == np.arange(NH)[None, None, :])
        .astype(ml_dtypes.bfloat16).transpose(1, 0, 2))          # [128,2,8]
    maskt_host = (hd_idx[None, :] == np.arange(NH)[:, None]).astype(np.float32)
    ident_host = np.eye(128, dtype=np.float32)
    wu_pad = np.zeros((VPAD, H), dtype=np.float32)
    wu_pad[:V] = wu

    in_maps = []
    for core in range(NCORES):
        tb = tok[core]
        idx_t = np.tile(np.ascontiguousarray(tb.reshape(SEQ // 16, 16).T),
                        (8, 1))                                  # [128, 128]
        wu_host = _part_chunk(
            np.ascontiguousarray(
                wu_pad[VSLICE * core: VSLICE * (core + 1)].T)).astype(
                    ml_dtypes.bfloat16)
        in_maps.append({
            "emb": emb_host, "idxs": idx_t, "pe": peT,
            "wq": wq_host, "wk": wk_host, "wv": wv_host, "wo": wo_host,
            "maskc": maskc_host, "maskt": maskt_host, "ident": ident_host,
            "wu": wu_host,
        })
    return in_maps


def get_nc():
    if "nc" not in _CACHE:
        _CACHE["nc"] = _build()
    return _CACHE["nc"]


def assemble(results):
    full = np.concatenate([results[i]["out"] for i in range(NCORES)],
                          axis=1)[:, :V]
    return np.ascontiguousarray(full.astype(np.float32))


def kernel(x, emb_w, wq, wk, wv, wo, wu):
    nc = get_nc()
    in_maps = prepare_in_maps(x, emb_w, wq, wk, wv, wo, wu)
    res = bass_utils.run_bass_kernel_spmd(
        nc, in_maps, core_ids=list(range(NCORES)))
    return assemble(res.results)

